# revision 3
# baseline (speedup 1.0000x reference)
"""Trainium2 Bass kernel v2 for DifferentiableGMM log-likelihood.

Computes  out[n] = logsumexp_k( -0.5*||(x[n]-mu[k])/s[k]||^2 - log|s[k]| + log w[k] )
for N=2,000,000 points, K=16 diagonal-covariance components, D=3.

v2 strategy (vs v1): eliminate the on-device feature transpose entirely.
  The host ships x already transposed into "contraction-row" layout
  (pure layout: reshape/cast, no host compute beyond the baseline's cast):
    xt [64, 16384] fp16 per core, row 4g+d = x4[16j+g, d], j in [0,16384)
  The device builds the quadratic feature rows with ONE tensor_mul:
    ft [128, cols]: rows 0..63 = xt*xt (squares), rows 64..127 = xt
  Per-point component log-probs come from two 128-contraction matmuls
  (pass P covers components 8P..8P+7):
    m[16t+c... out[8t+c, col] = sum_d A[k,d] x_d^2 + B[k,d] x_d,  k = 8P+c
  exp with the +c_k bias runs on ACT (table exp, bias arg) for some
  (pass, block-pair) units and on DVE (Schraudolph int16 bit-trick) for
  the rest, balancing the two engines.  The k-sum is a windowed
  ones-matmul accumulating 16 rounds (8 blocks x 2 passes) into one
  [128, 512] PSUM tile; one Ln pass emits the result.
"""

import os
import numpy as np

K = 16
D = 3
EPS = 1e-6
N_CORES = 8
N_FULL = 2_000_000

T_S = 4                      # sums-tiles per core
COLS_PER_S = 4096            # 16-point columns per sums-tile
COLS = T_S * COLS_PER_S      # 16384 columns per core
NPC = COLS * 16              # 262144 points per core
N_PAD = N_CORES * NPC        # 2097152

_compiled_cache = {}


def _schr_set():
    n = int(os.environ.get("GMM2_SCHR", "13"))
    return {round(i * 32 / n) % 32 for i in range(n)} if n else set()


def _build_nc(use_f32r=True):
    # Force the ACT-table chooser to the set holding Exp, Ln AND Copy
    # together so no table reloads happen mid-kernel.
    import concourse.bacc as _bacc_mod
    from concourse.hw_specs import get_activation_tables as _orig_gat
    def _only_combined(arch, __orig=_orig_gat):
        return {name: (fns if name == "natural_log_exp_and_others" else set())
                for name, fns in __orig(arch).items()}
    _bacc_mod.get_activation_tables = _only_combined

    reps = int(os.environ.get("GMM_REPS", "1"))
    import concourse.bacc as bacc
    import concourse.mybir as mybir
    import concourse.tile as tile
    from concourse._compat import get_trn_type

    f32 = mybir.dt.float32
    fp16 = mybir.dt.float16
    bf16 = mybir.dt.bfloat16
    i16 = mybir.dt.int16
    AF = mybir.ActivationFunctionType

    schr_set = _schr_set()
    ft_bufs = int(os.environ.get("GMM2_FTB", "3"))
    e_bufs = int(os.environ.get("GMM2_EB", "6"))
    m_bufs = int(os.environ.get("GMM2_MB", "3"))
    s_bufs = int(os.environ.get("GMM2_SB", "2"))
    o_bufs = int(os.environ.get("GMM2_OB", "3"))

    nc = bacc.Bacc(
        get_trn_type() or "TRN2",
        target_bir_lowering=False,
        debug=False,
        num_devices=N_CORES,
    )

    xt_dram = nc.dram_tensor("xt", [64, COLS], fp16, kind="ExternalInput")
    w_dram = nc.dram_tensor("wmat", [128, 4, 128], fp16, kind="ExternalInput")
    cvec_dram = nc.dram_tensor("cvec", [128, 4], f32, kind="ExternalInput")
    ones_dram = nc.dram_tensor("ones16", [128, 256], bf16, kind="ExternalInput")
    out_dram = nc.dram_tensor("out", [NPC], f32, kind="ExternalOutput")

    with tile.TileContext(nc) as tc:
        with (
            tc.tile_pool(name="singles", bufs=1) as singles,
            tc.tile_pool(name="ft", bufs=ft_bufs) as ft_pool,
            tc.tile_pool(name="etile", bufs=e_bufs) as e_pool,
            tc.tile_pool(name="osb", bufs=o_bufs) as out_pool,
            tc.tile_pool(name="mpsum", bufs=m_bufs, space="PSUM") as m_pool,
            tc.tile_pool(name="spsum", bufs=s_bufs, space="PSUM") as s_pool,
        ):
            # Constants, staged through compute-engine copies so consumers'
            # waits merge into existing engine sem domains.
            w_st = singles.tile([128, 4, 128], fp16)
            cvec_st = singles.tile([128, 4], f32)
            ones_st = singles.tile([128, 256], bf16)
            nc.sync.dma_start(w_st[:], w_dram[:, :, :])
            nc.sync.dma_start(cvec_st[:], cvec_dram[:, :])
            nc.sync.dma_start(ones_st[:], ones_dram[:, :])
            wmat = singles.tile([128, 4, 128], fp16)    # [p, {W0,W1,W0s,W1s}, col]
            cvec = singles.tile([128, 4], f32)          # cols: c0, c1, c2_0, c2_1
            ones16 = singles.tile([128, 256], bf16)
            nc.vector.tensor_copy(wmat[:], w_st[:])
            nc.vector.tensor_copy(ones16[:], ones_st[:])
            nc.scalar.copy(cvec[:], cvec_st[:])

            xt_view = xt_dram.ap().rearrange("p (s c) -> s p c", s=T_S)
            out_view = out_dram.ap().rearrange("(s p f) -> s p f", s=T_S, p=128)

            LAG = int(os.environ.get("GMM2_LAG", "2"))
            LAG_LN = int(os.environ.get("GMM2_LAG_LN", "1"))
            SQ_AHEAD = int(os.environ.get("GMM2_SQA", "2"))
            NU = T_S * 8  # units per iteration

            def main_body():
                # Per-iteration state; unit u covers cols [1024u, 1024u+1024)
                # of the per-core stream: S = u//8, pass P = (u//4)%2,
                # block-pair q = u%4 -> ft cols [1024*(u%8 rotated)]...
                # Simpler: within sums-tile S, local unit v=u%8: P=v//4,
                # q=v%4 covers ft[S] cols [1024q, 1024q+1024).
                fts = {}
                e_aps = {}
                ms = {}
                sums_tiles = {}
                lns = []

                def ensure_ft(S):
                    if S in fts or S >= T_S:
                        return
                    ft = ft_pool.tile([128, COLS_PER_S], fp16)
                    nc.sync.dma_start(ft[64:128, :], xt_view[S])
                    fts[S] = ft

                def do_square(u):
                    # squares for the ft cols unit u consumes
                    if u >= NU:
                        return
                    S, v = u // 8, u % 8
                    q = v % 4
                    ensure_ft(S)
                    ft = fts[S]
                    if v // 4 == 0:  # only once per (S, q): pass 0 does it
                        nc.vector.tensor_mul(
                            ft[0:64, 1024 * q:1024 * q + 1024],
                            ft[64:128, 1024 * q:1024 * q + 1024],
                            ft[64:128, 1024 * q:1024 * q + 1024])

                def do_mm_exp(u):
                    S, v = u // 8, u % 8
                    P, q = v // 4, v % 4
                    ft = fts[S]
                    schr = u in schr_set
                    w_ap = wmat[:, (P + 2) if schr else P, :]
                    m = m_pool.tile([128, 1024], f32)
                    for h in range(2):
                        nc.tensor.matmul(
                            m[:, 512 * h:512 * h + 512],
                            w_ap,
                            ft[:, 1024 * q + 512 * h:1024 * q + 512 * h + 512],
                            start=True, stop=True)
                    if schr:
                        e16 = e_pool.tile([128, 1024], i16, tag="e16")
                        nc.vector.tensor_scalar(
                            e16[:], m[:], cvec[:, (P + 2):(P + 3)],
                            0.0, mybir.AluOpType.add, mybir.AluOpType.max)
                        e_aps[u] = e16[:].bitcast(bf16)
                    else:
                        e = e_pool.tile([128, 1024], bf16, tag="ebf")
                        nc.scalar.activation(
                            e[:], m[:], AF.Exp,
                            bias=cvec[:, P:P + 1], scale=1.0)
                        e_aps[u] = e[:]

                def do_ksum(u):
                    S, v = u // 8, u % 8
                    q = v % 4
                    if S not in sums_tiles:
                        sums_tiles[S] = [s_pool.tile([128, 512], f32,
                                                     name="sums"), 0]
                    st = sums_tiles[S]
                    e_ap = e_aps.pop(u)
                    for h in range(2):
                        blk = 2 * q + h
                        nc.tensor.matmul(
                            st[0][:],
                            ones16[:, 120 - 16 * blk:248 - 16 * blk],
                            e_ap[:, 512 * h:512 * h + 512],
                            start=(st[1] == 0), stop=(st[1] == 15))
                        st[1] += 1
                    if st[1] == 16:
                        lns.append(S)

                def do_ln(S):
                    out_sb = out_pool.tile([128, 512], f32)
                    nc.scalar.activation(out_sb[:], sums_tiles[S][0][:], AF.Ln)
                    nc.sync.dma_start(out_view[S], out_sb[:])

                FTA = int(os.environ.get("GMM2_FTA", "8"))
                ensure_ft(0)
                for w in range(SQ_AHEAD):
                    do_square(w)
                pend_ln = []
                for g in range(NU + LAG):
                    ensure_ft((g + FTA) // 8)
                    if g < NU:
                        do_square(g + SQ_AHEAD)
                        do_mm_exp(g)
                    if g >= LAG:
                        do_ksum(g - LAG)
                        while lns:
                            pend_ln.append((lns.pop(0), g))
                    while pend_ln and (g - pend_ln[0][1] >= LAG_LN
                                       or g == NU + LAG - 1):
                        do_ln(pend_ln.pop(0)[0])
                while pend_ln:
                    do_ln(pend_ln.pop(0)[0])

            if reps == 1:
                main_body()
            elif os.environ.get("GMM2_NOHWLOOP", "0") == "1":
                for _ in range(reps):
                    main_body()
            else:
                # Unroll U bodies per hardware-loop iteration: For_i ends
                # every iteration with an all-engine barrier, so adjacent
                # bodies only pipeline inside one iteration.  U amortizes
                # the barrier + pipeline fill/drain cost.  With
                # GMM2_STAG=1, staggered semaphore resets replace the
                # all-engine barrier (body split into 4 reset stages) so
                # iterations pipeline through the back-edge.
                U = int(os.environ.get("GMM2_U", "4"))
                while reps % U:
                    U -= 1
                stag = (os.environ.get("GMM2_STAG", "0") == "1"
                        and U % 4 == 0)
                with tc.For_i(0, reps // U, 1, staggered_reset=stag):
                    for i in range(U):
                        if stag and i and i % (U // 4) == 0:
                            tc.stage_boundary()
                        main_body()

    nc.compile()
    return nc


def _output_permutation():
    """n[l]: point index for each linear output position l (per core)."""
    SS, qq, ff = np.meshgrid(np.arange(T_S), np.arange(128), np.arange(512),
                             indexing="ij")
    blk, t = qq // 16, qq % 16
    n = 16 * (SS * COLS_PER_S + blk * 512 + ff) + t
    return n.reshape(-1)


def _host_constants(means, covariances, weights):
    """wmat [128,4,128] (W0,W1,W0s,W1s), cvec [128,4], ones16 [128,256]."""
    covp = covariances.astype(np.float64) + EPS
    mu = means.astype(np.float64)
    A = -0.5 / covp                              # [K,D] coeff of x^2
    B = mu / covp                                # [K,D] coeff of x
    c_k = (-0.5 * (mu * mu / covp).sum(1) - 0.5 * np.log(covp).sum(1)
           - 0.5 * D * np.log(2 * np.pi) + np.log(weights.astype(np.float64)))

    Sc = 128.0 / np.log(2.0)
    schr_off = float(os.environ.get("GMM_SCHR_OFF", "5.5"))

    def build_w(P, scaled):
        w = np.zeros((128, 128), np.float64)
        for t in range(16):
            for c in range(8):
                k = 8 * P + c
                col = 8 * t + c
                w[4 * t:4 * t + 3, col] = A[k]
                w[64 + 4 * t:64 + 4 * t + 3, col] = B[k]
        if scaled:
            w = w * Sc
        return w.astype(np.float32)

    wmat = np.stack([build_w(0, False), build_w(1, False),
                     build_w(0, True), build_w(1, True)], axis=1)

    cvec = np.zeros((128, 4), np.float64)
    c2_k = c_k * Sc + 127.0 * 128.0 - schr_off
    for p in range(128):
        c = p % 8
        cvec[p, 0] = c_k[c]
        cvec[p, 1] = c_k[8 + c]
        cvec[p, 2] = c2_k[c]
        cvec[p, 3] = c2_k[8 + c]

    ones16 = np.zeros((128, 256), np.float32)
    for t in range(16):
        ones16[8 * t:8 * t + 8, 120 + t] = 1.0
    return wmat.astype(np.float32), cvec.astype(np.float32), ones16


def _prep_in_maps(x_pad, means, covariances, weights):
    """Per-core input maps. x_pad: [N_PAD, D] fp32."""
    import ml_dtypes
    wmat, cvec, ones16 = _host_constants(means, covariances, weights)
    wmat16 = wmat.astype(np.float16)
    ones_bf = ones16.astype(ml_dtypes.bfloat16)

    # host-side layout: [N_PAD, 3] -> per-core [64, COLS] fp16 where
    # row 4g+d = x4[16j+g, d] (x4 = x padded with a 4th lane of 1s)
    x4 = np.empty((N_PAD, 4), dtype=np.float16)
    x4[:, 0:3] = x_pad.astype(np.float16)
    x4[:, 3] = 1.0
    # [N_CORES, COLS, 16, 4] -> [N_CORES, 16, 4, COLS] -> [N_CORES, 64, COLS]
    xt = np.ascontiguousarray(
        x4.reshape(N_CORES, COLS, 16, 4).transpose(0, 2, 3, 1)
    ).reshape(N_CORES, 64, COLS)

    in_maps = []
    for c in range(N_CORES):
        in_maps.append({
            "xt": xt[c],
            "wmat": wmat16,
            "cvec": cvec,
            "ones16": ones_bf,
        })
    return in_maps


def kernel(x, means, covariances, weights):
    from concourse.bass_utils import run_bass_kernel_spmd

    x = np.ascontiguousarray(np.asarray(x, dtype=np.float32))
    means = np.ascontiguousarray(np.asarray(means, dtype=np.float32))
    covariances = np.ascontiguousarray(np.asarray(covariances, dtype=np.float32))
    weights = np.ascontiguousarray(np.asarray(weights, dtype=np.float32)).reshape(K)

    n = x.shape[0]
    x_pad = np.zeros((N_PAD, D), dtype=np.float32)
    x_pad[:n] = x

    key = "nc"
    if key not in _compiled_cache:
        _compiled_cache[key] = _build_nc(use_f32r=True)
    nc = _compiled_cache[key]

    in_maps = _prep_in_maps(x_pad, means, covariances, weights)

    res = run_bass_kernel_spmd(
        nc, in_maps, core_ids=list(range(N_CORES)),
        trace=bool(int(os.environ.get("GMM_TRACE", "0"))),
    )
    kernel.last_results = res

    perm = _output_permutation()
    out_pad = np.empty(N_PAD, dtype=np.float32)
    for c in range(N_CORES):
        raw = res.results[c]["out"].reshape(-1)
        out_pad[c * NPC + perm] = raw
    return out_pad[:n]


# revision 4
# speedup vs baseline: 1.0426x; 1.0426x over previous
"""Trainium2 Bass kernel v2 for DifferentiableGMM log-likelihood.

Computes  out[n] = logsumexp_k( -0.5*||(x[n]-mu[k])/s[k]||^2 - log|s[k]| + log w[k] )
for N=2,000,000 points, K=16 diagonal-covariance components, D=3.

v2 strategy (vs v1): eliminate the on-device feature transpose entirely.
  The host ships x already transposed into "contraction-row" layout
  (pure layout: reshape/cast, no host compute beyond the baseline's cast):
    xt [64, 16384] fp16 per core, row 4g+d = x4[16j+g, d], j in [0,16384)
  The device builds the quadratic feature rows with ONE tensor_mul:
    ft [128, cols]: rows 0..63 = xt*xt (squares), rows 64..127 = xt
  Per-point component log-probs come from two 128-contraction matmuls
  (pass P covers components 8P..8P+7):
    m[16t+c... out[8t+c, col] = sum_d A[k,d] x_d^2 + B[k,d] x_d,  k = 8P+c
  exp with the +c_k bias runs on ACT (table exp, bias arg) for some
  (pass, block-pair) units and on DVE (Schraudolph int16 bit-trick) for
  the rest, balancing the two engines.  The k-sum is a windowed
  ones-matmul accumulating 16 rounds (8 blocks x 2 passes) into one
  [128, 512] PSUM tile; one Ln pass emits the result.
"""

import os
import numpy as np

K = 16
D = 3
EPS = 1e-6
N_CORES = 8
N_FULL = 2_000_000

T_S = 4                      # sums-tiles per core
COLS_PER_S = 4096            # 16-point columns per sums-tile
COLS = T_S * COLS_PER_S      # 16384 columns per core
NPC = COLS * 16              # 262144 points per core
N_PAD = N_CORES * NPC        # 2097152

_compiled_cache = {}


def _schr_set():
    n = int(os.environ.get("GMM2_SCHR", "13"))
    return {round(i * 32 / n) % 32 for i in range(n)} if n else set()


def _build_nc(use_f32r=True):
    # Force the ACT-table chooser to the set holding Exp, Ln AND Copy
    # together so no table reloads happen mid-kernel.
    import concourse.bacc as _bacc_mod
    from concourse.hw_specs import get_activation_tables as _orig_gat
    def _only_combined(arch, __orig=_orig_gat):
        return {name: (fns if name == "natural_log_exp_and_others" else set())
                for name, fns in __orig(arch).items()}
    _bacc_mod.get_activation_tables = _only_combined

    reps = int(os.environ.get("GMM_REPS", "1"))
    import concourse.bacc as bacc
    import concourse.mybir as mybir
    import concourse.tile as tile
    from concourse._compat import get_trn_type

    f32 = mybir.dt.float32
    fp16 = mybir.dt.float16
    bf16 = mybir.dt.bfloat16
    i16 = mybir.dt.int16
    AF = mybir.ActivationFunctionType

    schr_set = _schr_set()
    ft_bufs = int(os.environ.get("GMM2_FTB", "3"))
    e_bufs = int(os.environ.get("GMM2_EB", "6"))
    m_bufs = int(os.environ.get("GMM2_MB", "3"))
    s_bufs = int(os.environ.get("GMM2_SB", "2"))
    o_bufs = int(os.environ.get("GMM2_OB", "3"))

    nc = bacc.Bacc(
        get_trn_type() or "TRN2",
        target_bir_lowering=False,
        debug=False,
        num_devices=N_CORES,
    )

    xt_dram = nc.dram_tensor("xt", [64, COLS], fp16, kind="ExternalInput")
    w_dram = nc.dram_tensor("wmat", [128, 4, 128], fp16, kind="ExternalInput")
    cvec_dram = nc.dram_tensor("cvec", [128, 4], f32, kind="ExternalInput")
    ones_dram = nc.dram_tensor("ones16", [128, 256], bf16, kind="ExternalInput")
    out_dram = nc.dram_tensor("out", [NPC], f32, kind="ExternalOutput")

    with tile.TileContext(nc) as tc:
        with (
            tc.tile_pool(name="singles", bufs=1) as singles,
            tc.tile_pool(name="ft", bufs=ft_bufs) as ft_pool,
            tc.tile_pool(name="etile", bufs=e_bufs) as e_pool,
            tc.tile_pool(name="osb", bufs=o_bufs) as out_pool,
            tc.tile_pool(name="mpsum", bufs=m_bufs, space="PSUM") as m_pool,
            tc.tile_pool(name="spsum", bufs=s_bufs, space="PSUM") as s_pool,
        ):
            # Constants, staged through compute-engine copies so consumers'
            # waits merge into existing engine sem domains.
            w_st = singles.tile([128, 4, 128], fp16)
            cvec_st = singles.tile([128, 4], f32)
            ones_st = singles.tile([128, 256], bf16)
            nc.sync.dma_start(w_st[:], w_dram[:, :, :])
            nc.sync.dma_start(cvec_st[:], cvec_dram[:, :])
            nc.sync.dma_start(ones_st[:], ones_dram[:, :])
            wmat = singles.tile([128, 4, 128], fp16)    # [p, {W0,W1,W0s,W1s}, col]
            cvec = singles.tile([128, 4], f32)          # cols: c0, c1, c2_0, c2_1
            ones16 = singles.tile([128, 256], bf16)
            nc.vector.tensor_copy(wmat[:], w_st[:])
            nc.vector.tensor_copy(ones16[:], ones_st[:])
            nc.scalar.copy(cvec[:], cvec_st[:])

            xt_view = xt_dram.ap().rearrange("p (s c) -> s p c", s=T_S)
            out_view = out_dram.ap().rearrange("(s p f) -> s p f", s=T_S, p=128)

            LAG = int(os.environ.get("GMM2_LAG", "2"))
            LAG_LN = int(os.environ.get("GMM2_LAG_LN", "1"))
            SQ_AHEAD = int(os.environ.get("GMM2_SQA", "2"))

            def main_body(n_bodies=1):
                NU = n_bodies * T_S * 8  # flattened units
                NS = n_bodies * T_S
                # Per-iteration state; unit u covers cols [1024u, 1024u+1024)
                # of the per-core stream: S = u//8, pass P = (u//4)%2,
                # block-pair q = u%4 -> ft cols [1024*(u%8 rotated)]...
                # Simpler: within sums-tile S, local unit v=u%8: P=v//4,
                # q=v%4 covers ft[S] cols [1024q, 1024q+1024).
                fts = {}
                e_aps = {}
                ms = {}
                sums_tiles = {}
                lns = []

                def ensure_ft(S):
                    if S in fts or S >= NS:
                        return
                    ft = ft_pool.tile([128, COLS_PER_S], fp16)
                    nc.sync.dma_start(ft[64:128, :], xt_view[S % T_S])
                    fts[S] = ft

                def do_square(u):
                    # squares for the ft cols unit u consumes
                    if u >= NU:
                        return
                    S, v = u // 8, u % 8
                    q = v % 4
                    ensure_ft(S)
                    ft = fts[S]
                    if v // 4 == 0:  # only once per (S, q): pass 0 does it
                        nc.vector.tensor_mul(
                            ft[0:64, 1024 * q:1024 * q + 1024],
                            ft[64:128, 1024 * q:1024 * q + 1024],
                            ft[64:128, 1024 * q:1024 * q + 1024])

                def do_mm_exp(u):
                    S, v = u // 8, u % 8
                    P, q = v // 4, v % 4
                    ft = fts[S]
                    schr = (u % 32) in schr_set
                    w_ap = wmat[:, (P + 2) if schr else P, :]
                    m = m_pool.tile([128, 1024], f32)
                    for h in range(2):
                        nc.tensor.matmul(
                            m[:, 512 * h:512 * h + 512],
                            w_ap,
                            ft[:, 1024 * q + 512 * h:1024 * q + 512 * h + 512],
                            start=True, stop=True)
                    if schr:
                        e16 = e_pool.tile([128, 1024], i16, tag="e16")
                        nc.vector.tensor_scalar(
                            e16[:], m[:], cvec[:, (P + 2):(P + 3)],
                            0.0, mybir.AluOpType.add, mybir.AluOpType.max)
                        e_aps[u] = e16[:].bitcast(bf16)
                    else:
                        e = e_pool.tile([128, 1024], bf16, tag="ebf")
                        nc.scalar.activation(
                            e[:], m[:], AF.Exp,
                            bias=cvec[:, P:P + 1], scale=1.0)
                        e_aps[u] = e[:]

                def do_ksum(u):
                    S, v = u // 8, u % 8
                    q = v % 4
                    if S not in sums_tiles:
                        sums_tiles[S] = [s_pool.tile([128, 512], f32,
                                                     name="sums"), 0]
                    st = sums_tiles[S]
                    e_ap = e_aps.pop(u)
                    for h in range(2):
                        blk = 2 * q + h
                        nc.tensor.matmul(
                            st[0][:],
                            ones16[:, 120 - 16 * blk:248 - 16 * blk],
                            e_ap[:, 512 * h:512 * h + 512],
                            start=(st[1] == 0), stop=(st[1] == 15))
                        st[1] += 1
                    if st[1] == 16:
                        lns.append(S)

                def do_ln(S):
                    out_sb = out_pool.tile([128, 512], f32)
                    nc.scalar.activation(out_sb[:], sums_tiles[S][0][:], AF.Ln)
                    nc.sync.dma_start(out_view[S % T_S], out_sb[:])
                    del sums_tiles[S]
                    fts.pop(S, None)

                FTA = int(os.environ.get("GMM2_FTA", "8"))
                ensure_ft(0)
                for w in range(SQ_AHEAD):
                    do_square(w)
                pend_ln = []
                for g in range(NU + LAG):
                    ensure_ft((g + FTA) // 8)
                    if g < NU:
                        do_square(g + SQ_AHEAD)
                        do_mm_exp(g)
                    if g >= LAG:
                        do_ksum(g - LAG)
                        while lns:
                            pend_ln.append((lns.pop(0), g))
                    while pend_ln and (g - pend_ln[0][1] >= LAG_LN
                                       or g == NU + LAG - 1):
                        do_ln(pend_ln.pop(0)[0])
                while pend_ln:
                    do_ln(pend_ln.pop(0)[0])

            if reps == 1:
                main_body(1)
            elif os.environ.get("GMM2_NOHWLOOP", "0") == "1":
                for _ in range(reps):
                    main_body(1)
            else:
                # Unroll U bodies per hardware-loop iteration: For_i ends
                # every iteration with an all-engine barrier, so adjacent
                # bodies only pipeline inside one iteration.  U amortizes
                # the barrier + pipeline fill/drain cost.  With
                # GMM2_STAG=1, staggered semaphore resets replace the
                # all-engine barrier (body split into 4 reset stages) so
                # iterations pipeline through the back-edge.
                U = int(os.environ.get("GMM2_U", "4"))
                while reps % U:
                    U -= 1
                with tc.For_i(0, reps // U, 1):
                    main_body(U)

    nc.compile()
    return nc


def _output_permutation():
    """n[l]: point index for each linear output position l (per core)."""
    SS, qq, ff = np.meshgrid(np.arange(T_S), np.arange(128), np.arange(512),
                             indexing="ij")
    blk, t = qq // 16, qq % 16
    n = 16 * (SS * COLS_PER_S + blk * 512 + ff) + t
    return n.reshape(-1)


def _host_constants(means, covariances, weights):
    """wmat [128,4,128] (W0,W1,W0s,W1s), cvec [128,4], ones16 [128,256]."""
    covp = covariances.astype(np.float64) + EPS
    mu = means.astype(np.float64)
    A = -0.5 / covp                              # [K,D] coeff of x^2
    B = mu / covp                                # [K,D] coeff of x
    c_k = (-0.5 * (mu * mu / covp).sum(1) - 0.5 * np.log(covp).sum(1)
           - 0.5 * D * np.log(2 * np.pi) + np.log(weights.astype(np.float64)))

    Sc = 128.0 / np.log(2.0)
    schr_off = float(os.environ.get("GMM_SCHR_OFF", "5.5"))

    def build_w(P, scaled):
        w = np.zeros((128, 128), np.float64)
        for t in range(16):
            for c in range(8):
                k = 8 * P + c
                col = 8 * t + c
                w[4 * t:4 * t + 3, col] = A[k]
                w[64 + 4 * t:64 + 4 * t + 3, col] = B[k]
        if scaled:
            w = w * Sc
        return w.astype(np.float32)

    wmat = np.stack([build_w(0, False), build_w(1, False),
                     build_w(0, True), build_w(1, True)], axis=1)

    cvec = np.zeros((128, 4), np.float64)
    c2_k = c_k * Sc + 127.0 * 128.0 - schr_off
    for p in range(128):
        c = p % 8
        cvec[p, 0] = c_k[c]
        cvec[p, 1] = c_k[8 + c]
        cvec[p, 2] = c2_k[c]
        cvec[p, 3] = c2_k[8 + c]

    ones16 = np.zeros((128, 256), np.float32)
    for t in range(16):
        ones16[8 * t:8 * t + 8, 120 + t] = 1.0
    return wmat.astype(np.float32), cvec.astype(np.float32), ones16


def _prep_in_maps(x_pad, means, covariances, weights):
    """Per-core input maps. x_pad: [N_PAD, D] fp32."""
    import ml_dtypes
    wmat, cvec, ones16 = _host_constants(means, covariances, weights)
    wmat16 = wmat.astype(np.float16)
    ones_bf = ones16.astype(ml_dtypes.bfloat16)

    # host-side layout: [N_PAD, 3] -> per-core [64, COLS] fp16 where
    # row 4g+d = x4[16j+g, d] (x4 = x padded with a 4th lane of 1s)
    x4 = np.empty((N_PAD, 4), dtype=np.float16)
    x4[:, 0:3] = x_pad.astype(np.float16)
    x4[:, 3] = 1.0
    # [N_CORES, COLS, 16, 4] -> [N_CORES, 16, 4, COLS] -> [N_CORES, 64, COLS]
    xt = np.ascontiguousarray(
        x4.reshape(N_CORES, COLS, 16, 4).transpose(0, 2, 3, 1)
    ).reshape(N_CORES, 64, COLS)

    in_maps = []
    for c in range(N_CORES):
        in_maps.append({
            "xt": xt[c],
            "wmat": wmat16,
            "cvec": cvec,
            "ones16": ones_bf,
        })
    return in_maps


def kernel(x, means, covariances, weights):
    from concourse.bass_utils import run_bass_kernel_spmd

    x = np.ascontiguousarray(np.asarray(x, dtype=np.float32))
    means = np.ascontiguousarray(np.asarray(means, dtype=np.float32))
    covariances = np.ascontiguousarray(np.asarray(covariances, dtype=np.float32))
    weights = np.ascontiguousarray(np.asarray(weights, dtype=np.float32)).reshape(K)

    n = x.shape[0]
    x_pad = np.zeros((N_PAD, D), dtype=np.float32)
    x_pad[:n] = x

    key = "nc"
    if key not in _compiled_cache:
        _compiled_cache[key] = _build_nc(use_f32r=True)
    nc = _compiled_cache[key]

    in_maps = _prep_in_maps(x_pad, means, covariances, weights)

    res = run_bass_kernel_spmd(
        nc, in_maps, core_ids=list(range(N_CORES)),
        trace=bool(int(os.environ.get("GMM_TRACE", "0"))),
    )
    kernel.last_results = res

    perm = _output_permutation()
    out_pad = np.empty(N_PAD, dtype=np.float32)
    for c in range(N_CORES):
        raw = res.results[c]["out"].reshape(-1)
        out_pad[c * NPC + perm] = raw
    return out_pad[:n]


# revision 5
# speedup vs baseline: 1.1469x; 1.1000x over previous
"""Trainium2 Bass kernel v2 for DifferentiableGMM log-likelihood.

Computes  out[n] = logsumexp_k( -0.5*||(x[n]-mu[k])/s[k]||^2 - log|s[k]| + log w[k] )
for N=2,000,000 points, K=16 diagonal-covariance components, D=3.

v2 strategy (vs v1): eliminate the on-device feature transpose entirely.
  The host ships x already transposed into "contraction-row" layout
  (pure layout: reshape/cast, no host compute beyond the baseline's cast):
    xt [64, 16384] fp16 per core, row 4g+d = x4[16j+g, d], j in [0,16384)
  The device builds the quadratic feature rows with ONE tensor_mul:
    ft [128, cols]: rows 0..63 = xt*xt (squares), rows 64..127 = xt
  Per-point component log-probs come from two 128-contraction matmuls
  (pass P covers components 8P..8P+7):
    m[16t+c... out[8t+c, col] = sum_d A[k,d] x_d^2 + B[k,d] x_d,  k = 8P+c
  exp with the +c_k bias runs on ACT (table exp, bias arg) for some
  (pass, block-pair) units and on DVE (Schraudolph int16 bit-trick) for
  the rest, balancing the two engines.  The k-sum is a windowed
  ones-matmul accumulating 16 rounds (8 blocks x 2 passes) into one
  [128, 512] PSUM tile; one Ln pass emits the result.
"""

import os
import numpy as np

K = 16
D = 3
EPS = 1e-6
N_CORES = 8
N_FULL = 2_000_000

T_S = 4                      # sums-tiles per core
COLS_PER_S = 4096            # 16-point columns per sums-tile
COLS = T_S * COLS_PER_S      # 16384 columns per core
NPC = COLS * 16              # 262144 points per core
N_PAD = N_CORES * NPC        # 2097152

_compiled_cache = {}


def _schr_set():
    n = int(os.environ.get("GMM2_SCHR", "26"))
    return {round(i * 32 / n) % 32 for i in range(n)} if n else set()


def _fp8_set():
    n = int(os.environ.get("GMM2_FP8", "9"))
    return {round(i * 16 / n) % 16 for i in range(n)} if n else set()


def _build_nc(use_f32r=True):
    # Force the ACT-table chooser to the set holding Exp, Ln AND Copy
    # together so no table reloads happen mid-kernel.
    import concourse.bacc as _bacc_mod
    from concourse.hw_specs import get_activation_tables as _orig_gat
    def _only_combined(arch, __orig=_orig_gat):
        return {name: (fns if name == "natural_log_exp_and_others" else set())
                for name, fns in __orig(arch).items()}
    _bacc_mod.get_activation_tables = _only_combined

    reps = int(os.environ.get("GMM_REPS", "1"))
    import concourse.bacc as bacc
    import concourse.mybir as mybir
    import concourse.tile as tile
    from concourse._compat import get_trn_type

    f32 = mybir.dt.float32
    fp16 = mybir.dt.float16
    bf16 = mybir.dt.bfloat16
    i16 = mybir.dt.int16
    AF = mybir.ActivationFunctionType

    schr_set = _schr_set()
    ft_bufs = int(os.environ.get("GMM2_FTB", "3"))
    e_bufs = int(os.environ.get("GMM2_EB", "6"))
    m_bufs = int(os.environ.get("GMM2_MB", "3"))
    s_bufs = int(os.environ.get("GMM2_SB", "2"))
    o_bufs = int(os.environ.get("GMM2_OB", "3"))

    nc = bacc.Bacc(
        get_trn_type() or "TRN2",
        target_bir_lowering=False,
        debug=False,
        num_devices=N_CORES,
    )

    xt_dram = nc.dram_tensor("xt", [64, COLS], fp16, kind="ExternalInput")
    w_dram = nc.dram_tensor("wmat", [128, 4, 128], fp16, kind="ExternalInput")
    cvec_dram = nc.dram_tensor("cvec", [128, 4], f32, kind="ExternalInput")
    ones_dram = nc.dram_tensor("ones16", [128, 256], bf16, kind="ExternalInput")
    f8 = mybir.dt.float8e4
    ones8_dram = nc.dram_tensor("ones8", [128, 2, 256], f8, kind="ExternalInput")
    out_dram = nc.dram_tensor("out", [NPC], f32, kind="ExternalOutput")

    with tile.TileContext(nc) as tc:
        with (
            tc.tile_pool(name="singles", bufs=1) as singles,
            tc.tile_pool(name="ft", bufs=ft_bufs) as ft_pool,
            tc.tile_pool(name="etile", bufs=e_bufs) as e_pool,
            tc.tile_pool(name="osb", bufs=o_bufs) as out_pool,
            tc.tile_pool(name="mpsum", bufs=m_bufs, space="PSUM") as m_pool,
            tc.tile_pool(name="spsum", bufs=s_bufs, space="PSUM") as s_pool,
        ):
            # Constants, staged through compute-engine copies so consumers'
            # waits merge into existing engine sem domains.
            w_st = singles.tile([128, 4, 128], fp16)
            cvec_st = singles.tile([128, 4], f32)
            ones_st = singles.tile([128, 256], bf16)
            ones8_st = singles.tile([128, 2, 256], f8)
            nc.sync.dma_start(w_st[:], w_dram[:, :, :])
            nc.sync.dma_start(cvec_st[:], cvec_dram[:, :])
            nc.sync.dma_start(ones_st[:], ones_dram[:, :])
            nc.sync.dma_start(ones8_st[:], ones8_dram[:, :, :])
            wmat = singles.tile([128, 4, 128], fp16)    # [p, {W0,W1,W0s,W1s}, col]
            cvec = singles.tile([128, 4], f32)          # cols: c0, c1, c2_0, c2_1
            ones16 = singles.tile([128, 256], bf16)
            ones8 = singles.tile([128, 2, 256], f8)
            nc.vector.tensor_copy(wmat[:], w_st[:])
            nc.vector.tensor_copy(ones16[:], ones_st[:])
            nc.vector.tensor_copy(ones8[:], ones8_st[:])
            nc.scalar.copy(cvec[:], cvec_st[:])

            xt_view = xt_dram.ap().rearrange("p (s c) -> s p c", s=T_S)
            out_view = out_dram.ap().rearrange("(s p f) -> s p f", s=T_S, p=128)

            LAG = int(os.environ.get("GMM2_LAG", "2"))
            LAG_LN = int(os.environ.get("GMM2_LAG_LN", "1"))
            SQ_AHEAD = int(os.environ.get("GMM2_SQA", "2"))

            fp8_set = _fp8_set()

            def main_body(n_bodies=1):
                # Unit u: S = u//8, v = u%8, block q = v//2, pass P = v%2
                # (passes adjacent so an fp8 block's two planes are ready
                # back-to-back).  Block b = u//2 = 4S + q; fp8 blocks do a
                # paired DoubleRow k-sum (2 rounds), others 4 bf16 rounds.
                NU = n_bodies * T_S * 8  # flattened pass-units
                NS = n_bodies * T_S
                fts = {}
                e_aps = {}
                sums_tiles = {}
                lns = []

                def is_fp8(b):
                    return (b % 16) in fp8_set

                def rounds_of(S):
                    return sum(2 if is_fp8(4 * S + q) else 4 for q in range(4))

                def ensure_ft(S):
                    if S in fts or S >= NS:
                        return
                    ft = ft_pool.tile([128, COLS_PER_S], fp16)
                    nc.sync.dma_start(ft[64:128, :], xt_view[S % T_S])
                    fts[S] = ft

                def do_square(u):
                    # squares for the ft cols of unit u's block (pass 0 only)
                    if u >= NU:
                        return
                    S, v = u // 8, u % 8
                    q, P = v // 2, v % 2
                    ensure_ft(S)
                    ft = fts[S]
                    if P == 0:
                        nc.vector.tensor_mul(
                            ft[0:64, 1024 * q:1024 * q + 1024],
                            ft[64:128, 1024 * q:1024 * q + 1024],
                            ft[64:128, 1024 * q:1024 * q + 1024])

                def do_mm_exp(u):
                    S, v = u // 8, u % 8
                    q, P = v // 2, v % 2
                    b = u // 2
                    ft = fts[S]
                    fp8b = is_fp8(b)
                    schr = (not fp8b) and ((u % 32) in schr_set)
                    w_ap = wmat[:, (P + 2) if schr else P, :]
                    m = m_pool.tile([128, 1024], f32)
                    for h in range(2):
                        nc.tensor.matmul(
                            m[:, 512 * h:512 * h + 512],
                            w_ap,
                            ft[:, 1024 * q + 512 * h:1024 * q + 512 * h + 512],
                            start=True, stop=True)
                    if fp8b:
                        if P == 0:
                            e8 = e_pool.tile([128, 2, 1024], f8, tag="e8")
                            e_aps[b] = e8
                        e8 = e_aps[b]
                        nc.scalar.activation(
                            e8[:, P, :], m[:], AF.Exp,
                            bias=cvec[:, P:P + 1], scale=1.0)
                    elif schr:
                        e16 = e_pool.tile([128, 1024], i16, tag="e16")
                        nc.vector.tensor_scalar(
                            e16[:], m[:], cvec[:, (P + 2):(P + 3)],
                            0.0, mybir.AluOpType.add, mybir.AluOpType.max)
                        e_aps.setdefault(b, {})[P] = e16[:].bitcast(bf16)
                    else:
                        e = e_pool.tile([128, 1024], bf16, tag="ebf")
                        nc.scalar.activation(
                            e[:], m[:], AF.Exp,
                            bias=cvec[:, P:P + 1], scale=1.0)
                        e_aps.setdefault(b, {})[P] = e[:]

                def do_ksum_block(b):
                    S, q = b // 4, b % 4
                    if S not in sums_tiles:
                        sums_tiles[S] = [s_pool.tile([128, 512], f32,
                                                     name="sums"),
                                         0, rounds_of(S)]
                    st = sums_tiles[S]
                    e_ap = e_aps.pop(b)
                    if is_fp8(b):
                        for h in range(2):
                            blk = 2 * q + h
                            nc.tensor.matmul(
                                st[0][:],
                                ones8[:, :, 120 - 16 * blk:248 - 16 * blk],
                                e_ap[:, :, 512 * h:512 * h + 512],
                                start=(st[1] == 0), stop=(st[1] == st[2] - 1),
                                perf_mode=mybir.MatmulPerfMode.DoubleRow)
                            st[1] += 1
                    else:
                        for P in range(2):
                            for h in range(2):
                                blk = 2 * q + h
                                nc.tensor.matmul(
                                    st[0][:],
                                    ones16[:, 120 - 16 * blk:248 - 16 * blk],
                                    e_ap[P][:, 512 * h:512 * h + 512],
                                    start=(st[1] == 0),
                                    stop=(st[1] == st[2] - 1))
                                st[1] += 1
                    if st[1] == st[2]:
                        lns.append(S)

                def do_ln(S):
                    out_sb = out_pool.tile([128, 512], f32)
                    nc.scalar.activation(out_sb[:], sums_tiles[S][0][:], AF.Ln)
                    nc.sync.dma_start(out_view[S % T_S], out_sb[:])
                    del sums_tiles[S]
                    fts.pop(S, None)

                FTA = int(os.environ.get("GMM2_FTA", "8"))
                ensure_ft(0)
                for w in range(SQ_AHEAD):
                    do_square(w)
                pend_ln = []
                # ksum for block b runs at g = 2b + 1 + LAG (after both
                # passes' exps have had LAG units of slack)
                for g in range(NU + LAG + 1):
                    ensure_ft((g + FTA) // 8)
                    if g < NU:
                        do_square(g + SQ_AHEAD)
                        do_mm_exp(g)
                    gb = g - LAG - 1
                    if gb >= 0 and gb % 2 == 1:
                        do_ksum_block(gb // 2)
                        while lns:
                            pend_ln.append((lns.pop(0), g))
                    while pend_ln and (g - pend_ln[0][1] >= LAG_LN
                                       or g == NU + LAG):
                        do_ln(pend_ln.pop(0)[0])
                while pend_ln:
                    do_ln(pend_ln.pop(0)[0])

            if reps == 1:
                main_body(1)
            elif os.environ.get("GMM2_NOHWLOOP", "0") == "1":
                for _ in range(reps):
                    main_body(1)
            else:
                # Unroll U bodies per hardware-loop iteration: For_i ends
                # every iteration with an all-engine barrier, so adjacent
                # bodies only pipeline inside one iteration.  U amortizes
                # the barrier + pipeline fill/drain cost.  With
                # GMM2_STAG=1, staggered semaphore resets replace the
                # all-engine barrier (body split into 4 reset stages) so
                # iterations pipeline through the back-edge.
                U = int(os.environ.get("GMM2_U", "4"))
                while reps % U:
                    U -= 1
                with tc.For_i(0, reps // U, 1):
                    main_body(U)

    nc.compile()
    return nc


def _output_permutation():
    """n[l]: point index for each linear output position l (per core)."""
    SS, qq, ff = np.meshgrid(np.arange(T_S), np.arange(128), np.arange(512),
                             indexing="ij")
    blk, t = qq // 16, qq % 16
    n = 16 * (SS * COLS_PER_S + blk * 512 + ff) + t
    return n.reshape(-1)


def _host_constants(means, covariances, weights):
    """wmat [128,4,128] (W0,W1,W0s,W1s), cvec [128,4], ones16 [128,256]."""
    covp = covariances.astype(np.float64) + EPS
    mu = means.astype(np.float64)
    A = -0.5 / covp                              # [K,D] coeff of x^2
    B = mu / covp                                # [K,D] coeff of x
    c_k = (-0.5 * (mu * mu / covp).sum(1) - 0.5 * np.log(covp).sum(1)
           - 0.5 * D * np.log(2 * np.pi) + np.log(weights.astype(np.float64)))

    Sc = 128.0 / np.log(2.0)
    schr_off = float(os.environ.get("GMM_SCHR_OFF", "5.5"))

    def build_w(P, scaled):
        w = np.zeros((128, 128), np.float64)
        for t in range(16):
            for c in range(8):
                k = 8 * P + c
                col = 8 * t + c
                w[4 * t:4 * t + 3, col] = A[k]
                w[64 + 4 * t:64 + 4 * t + 3, col] = B[k]
        if scaled:
            w = w * Sc
        return w.astype(np.float32)

    wmat = np.stack([build_w(0, False), build_w(1, False),
                     build_w(0, True), build_w(1, True)], axis=1)

    cvec = np.zeros((128, 4), np.float64)
    c2_k = c_k * Sc + 127.0 * 128.0 - schr_off
    for p in range(128):
        c = p % 8
        cvec[p, 0] = c_k[c]
        cvec[p, 1] = c_k[8 + c]
        cvec[p, 2] = c2_k[c]
        cvec[p, 3] = c2_k[8 + c]

    ones16 = np.zeros((128, 256), np.float32)
    for t in range(16):
        ones16[8 * t:8 * t + 8, 120 + t] = 1.0
    ones8 = np.stack([ones16, ones16], axis=1)  # [128, 2, 256]
    return wmat.astype(np.float32), cvec.astype(np.float32), ones16, ones8


def _prep_in_maps(x_pad, means, covariances, weights):
    """Per-core input maps. x_pad: [N_PAD, D] fp32."""
    import ml_dtypes
    wmat, cvec, ones16, ones8 = _host_constants(means, covariances, weights)
    wmat16 = wmat.astype(np.float16)
    ones_bf = ones16.astype(ml_dtypes.bfloat16)
    ones8_f8 = ones8.astype(ml_dtypes.float8_e4m3)

    # host-side layout: [N_PAD, 3] -> per-core [64, COLS] fp16 where
    # row 4g+d = x4[16j+g, d] (x4 = x padded with a 4th lane of 1s)
    x4 = np.empty((N_PAD, 4), dtype=np.float16)
    x4[:, 0:3] = x_pad.astype(np.float16)
    x4[:, 3] = 1.0
    # [N_CORES, COLS, 16, 4] -> [N_CORES, 16, 4, COLS] -> [N_CORES, 64, COLS]
    xt = np.ascontiguousarray(
        x4.reshape(N_CORES, COLS, 16, 4).transpose(0, 2, 3, 1)
    ).reshape(N_CORES, 64, COLS)

    in_maps = []
    for c in range(N_CORES):
        in_maps.append({
            "xt": xt[c],
            "wmat": wmat16,
            "cvec": cvec,
            "ones16": ones_bf,
            "ones8": ones8_f8,
        })
    return in_maps


def kernel(x, means, covariances, weights):
    from concourse.bass_utils import run_bass_kernel_spmd

    x = np.ascontiguousarray(np.asarray(x, dtype=np.float32))
    means = np.ascontiguousarray(np.asarray(means, dtype=np.float32))
    covariances = np.ascontiguousarray(np.asarray(covariances, dtype=np.float32))
    weights = np.ascontiguousarray(np.asarray(weights, dtype=np.float32)).reshape(K)

    n = x.shape[0]
    x_pad = np.zeros((N_PAD, D), dtype=np.float32)
    x_pad[:n] = x

    key = "nc"
    if key not in _compiled_cache:
        _compiled_cache[key] = _build_nc(use_f32r=True)
    nc = _compiled_cache[key]

    in_maps = _prep_in_maps(x_pad, means, covariances, weights)

    res = run_bass_kernel_spmd(
        nc, in_maps, core_ids=list(range(N_CORES)),
        trace=bool(int(os.environ.get("GMM_TRACE", "0"))),
    )
    kernel.last_results = res

    perm = _output_permutation()
    out_pad = np.empty(N_PAD, dtype=np.float32)
    for c in range(N_CORES):
        raw = res.results[c]["out"].reshape(-1)
        out_pad[c * NPC + perm] = raw
    return out_pad[:n]
